# revision 8
# baseline (speedup 1.0000x reference)
# Trainium2 Bass kernel for nn_AttentionBlock (B=8, C=512, H=W=32, 8 heads).
#
# Sharding: pure data-parallel over batch B — each of the 8 NeuronCores gets one
# batch element and computes the full attention block for it (GroupNorm -> QKV ->
# 8-head attention over T=1024, dh=64 -> o_proj -> LayerNorm -> +residual).
# No collectives needed; outputs are concatenated on host.
#
# Self-contained: imports only installed packages (numpy, ml_dtypes, concourse).
import functools
import os

import ml_dtypes
import numpy as np

import concourse.bacc as bacc
import concourse.bass as bass
import concourse.mybir as mybir
import concourse.tile as tile
from concourse import bass_utils
from concourse.bass import ts

F32 = mybir.dt.float32
BF16 = mybir.dt.bfloat16
AF = mybir.ActivationFunctionType
OP = mybir.AluOpType

B, C, Hh, Ww = 8, 512, 32, 32
T = Hh * Ww          # 1024
NG = 32              # groupnorm groups
GSZ = C // NG        # 16 channels per group
NH = 8               # heads
DH = C // NH         # 64
EPS = 1e-5
SCALE = float(DH) ** -0.25   # applied to q only (folded into exp scale)
P = 128
NCC = C // P         # 4 channel chunks
NTT = T // P         # 8 token tiles

_last_result = None  # stash of BassKernelResults for test harness introspection


class _Bacc(bacc.Bacc):
    """Bacc that loads ONE activation-function table set covering every
    activation used (e.g. natural_log_exp_and_others for Exp+Ln), instead of
    the default greedy per-function choice which thrashes between the
    exp-only and ln-only sets (~2.7us per reload)."""

    def insert_act_table_loads(self):
        funcs = set()
        for b in self.main_func.blocks:
            for i in b.instructions:
                if isinstance(i, mybir.InstActivation):
                    funcs.add(i.func)
        if not funcs:
            return
        tables = bacc.get_activation_tables(self.m.arch)
        chosen = None
        for idx, (_name, fset) in enumerate(tables.items()):
            if funcs <= fset:
                chosen = idx
                break
        if chosen is None:
            return super().insert_act_table_loads()
        for b in self.main_func.blocks:
            for pos, i in enumerate(b.instructions):
                if isinstance(i, mybir.InstActivation):
                    load = mybir.InstLoadActFuncSet(
                        act_func_set_id=chosen,
                        name=self.get_next_instruction_name(),
                        ins=[], outs=[])
                    load.engine = i.engine
                    self.register_instruction(load)
                    b.instructions.insert(pos, load)
                    return


def _build(gn_affine: bool, qb: bool, kb: bool, vb: bool, ob: bool, ln_affine: bool):
    nc = _Bacc("TRN2", target_bir_lowering=False, debug=False)

    x_d = nc.dram_tensor("x", (C, T), F32, kind="ExternalInput")
    wq_d = nc.dram_tensor("wqT", (C, C), BF16, kind="ExternalInput")
    wk_d = nc.dram_tensor("wkT", (C, C), BF16, kind="ExternalInput")
    wv_d = nc.dram_tensor("wvT", (C, C), BF16, kind="ExternalInput")
    wo_d = nc.dram_tensor("woT", (C, C), BF16, kind="ExternalInput")
    g8_d = nc.dram_tensor("g8", (P, 8), F32, kind="ExternalInput")
    hm_d = nc.dram_tensor("hm", (8, P), F32, kind="ExternalInput")
    id_d = nc.dram_tensor("ident", (P, P), F32, kind="ExternalInput")
    gng_d = gnb_d = lng_d = lnb_d = None
    if gn_affine:
        gng_d = nc.dram_tensor("gn_gamma", (C,), F32, kind="ExternalInput")
        gnb_d = nc.dram_tensor("gn_beta", (C,), F32, kind="ExternalInput")
    if ln_affine:
        lng_d = nc.dram_tensor("ln_gamma", (C,), F32, kind="ExternalInput")
        lnb_d = nc.dram_tensor("ln_beta", (C,), F32, kind="ExternalInput")
    bias_d = {}
    for name, flag in (("bq", qb), ("bk", kb), ("bv", vb), ("bo", ob)):
        if flag:
            bias_d[name] = nc.dram_tensor(name, (1, C), BF16, kind="ExternalInput")
    out_d = nc.dram_tensor("out", (C, T), F32, kind="ExternalOutput")

    with tile.TileContext(nc) as tc:
        with tc.tile_pool(name="singles", bufs=1) as sg:
            # ---- persistent SBUF tiles ----
            x_sb = [sg.tile([P, T], F32, name=f"x{i}", tag=f"x{i}") for i in range(NCC)]
            xn_sb = [sg.tile([P, T], BF16, name=f"xn{i}", tag=f"xn{i}") for i in range(NCC)]
            wq_sb = [sg.tile([P, C], BF16, name=f"wq{i}", tag=f"wq{i}") for i in range(NCC)]
            wk_sb = [sg.tile([P, C], BF16, name=f"wk{i}", tag=f"wk{i}") for i in range(NCC)]
            wv_sb = [sg.tile([P, C], BF16, name=f"wv{i}", tag=f"wv{i}") for i in range(NCC)]
            wo_sb = [sg.tile([P, C], BF16, name=f"wo{i}", tag=f"wo{i}") for i in range(NCC)]
            qT_sb = [sg.tile([P, T], BF16, name=f"qT{i}", tag=f"qT{i}") for i in range(NCC)]
            kT_sb = [sg.tile([P, T], BF16, name=f"kT{i}", tag=f"kT{i}") for i in range(NCC)]
            v_sb = [sg.tile([P, C], BF16, name=f"v{i}", tag=f"v{i}") for i in range(NTT)]
            ctxT_sb = [sg.tile([P, T], BF16, name=f"ctxT{i}", tag=f"ctxT{i}") for i in range(NCC)]
            out_sb = [sg.tile([P, T], F32, name=f"o{i}", tag=f"o{i}") for i in range(NCC)]
            g8_sb = sg.tile([P, 8], F32, name="g8s", tag="g8s")
            hm_sb = sg.tile([8, P], F32, name="hms", tag="hms")
            id_sb = sg.tile([P, P], F32, name="ids", tag="ids")
            ones64 = sg.tile([P, DH], BF16, name="ones64", tag="ones64")
            onesrow = sg.tile([1, C], BF16, name="onesrow", tag="onesrow")

            # ---- input DMAs ----
            for i in range(NCC):
                nc.sync.dma_start(out=x_sb[i], in_=x_d[ts(i, P), :])
            for wsb, wd in ((wq_sb, wq_d), (wk_sb, wk_d), (wv_sb, wv_d), (wo_sb, wo_d)):
                for i in range(NCC):
                    nc.sync.dma_start(out=wsb[i], in_=wd[ts(i, P), :])
            nc.sync.dma_start(out=g8_sb, in_=g8_d[:, :])
            nc.sync.dma_start(out=hm_sb, in_=hm_d[:, :])
            nc.sync.dma_start(out=id_sb, in_=id_d[:, :])
            eps128 = sg.tile([P, 1], F32, name="eps128", tag="eps128")
            nc.gpsimd.memset(ones64, 1.0)
            nc.gpsimd.memset(onesrow, 1.0)
            nc.gpsimd.memset(eps128, EPS)

            gam_t = bet_t = None
            if gn_affine:
                gam_t = sg.tile([P, NCC], F32, name="gamt", tag="gamt")
                bet_t = sg.tile([P, NCC], F32, name="bett", tag="bett")
                nc.sync.dma_start(out=gam_t, in_=gng_d.rearrange("(t p) -> p t", p=P))
                nc.sync.dma_start(out=bet_t, in_=gnb_d.rearrange("(t p) -> p t", p=P))
            lng_bc = lnb_bc = None
            if ln_affine:
                lng_bc = sg.tile([P, C], F32, name="lngb", tag="lngb")
                lnb_bc = sg.tile([P, C], F32, name="lnbb", tag="lnbb")
                nc.gpsimd.dma_start(out=lng_bc, in_=bass.AP(
                    tensor=lng_d.tensor, offset=lng_d.offset, ap=[[0, P], [1, C]]))
                nc.gpsimd.dma_start(out=lnb_bc, in_=bass.AP(
                    tensor=lnb_d.tensor, offset=lnb_d.offset, ap=[[0, P], [1, C]]))
            bias_sb = {}
            for name in bias_d:
                bias_sb[name] = sg.tile([1, C], BF16, name=f"{name}s", tag=f"{name}s")
                nc.sync.dma_start(out=bias_sb[name], in_=bias_d[name][:, :])

            # ================= GroupNorm =================
            with tc.tile_pool(name="gn_ps", bufs=1, space="PSUM") as gnps, \
                 tc.tile_pool(name="gn_sb", bufs=1) as gnsb:
                st_ps = gnps.tile([8, 2 * NCC], F32, name="st_ps", tag="st_ps")
                bc_ps = gnps.tile([P, 8], F32, name="bc_ps", tag="bc_ps")
                for cc in range(NCC):
                    bn = gnsb.tile([P, 2, 6], F32, name=f"bn{cc}", tag="bn")
                    nc.vector.bn_stats(out=bn[:, 0, :], in_=x_sb[cc][:, 0:512])
                    nc.vector.bn_stats(out=bn[:, 1, :], in_=x_sb[cc][:, 512:1024])
                    mv = gnsb.tile([P, 2], F32, name=f"mv{cc}", tag="mv")
                    nc.vector.bn_aggr(out=mv, in_=bn)
                    # mv = (mu_p, var_p); turn col1 into E[x^2]_p = var + mu^2
                    tmp1 = gnsb.tile([P, 1], F32, name=f"tmp1_{cc}", tag="tmp1")
                    nc.vector.tensor_mul(tmp1, mv[:, 0:1], mv[:, 0:1])
                    nc.vector.tensor_add(mv[:, 1:2], mv[:, 1:2], tmp1)
                    # group-mean over 16-partition groups: [8, 2] per chunk
                    nc.tensor.matmul(st_ps[:, 2 * cc:2 * cc + 2], lhsT=g8_sb, rhs=mv,
                                     start=True, stop=True)
                gs = gnsb.tile([8, 2 * NCC], F32, name="gs", tag="gs")
                nc.vector.tensor_copy(gs, st_ps)
                gsv = gs.rearrange("p (t s) -> p t s", s=2)
                mug = gsv[:, :, 0]          # [8, 4] strided
                e2g = gsv[:, :, 1]
                tmp4 = gnsb.tile([8, NCC], F32, name="tmp4", tag="tmp4")
                nc.vector.tensor_mul(tmp4, mug, mug)
                varg = gnsb.tile([8, NCC], F32, name="varg", tag="varg")
                nc.vector.tensor_sub(varg, e2g, tmp4)
                # rstd = exp(-0.5 * ln(var + eps))
                lnv = gnsb.tile([8, NCC], F32, name="lnv", tag="lnv")
                nc.scalar.activation(lnv, varg, AF.Ln, bias=eps128[0:8, :])
                rstdg = gnsb.tile([8, NCC], F32, name="rstdg", tag="rstdg")
                nc.scalar.activation(rstdg, lnv, AF.Exp, scale=-0.5)
                small8 = gnsb.tile([8, 2 * NCC], F32, name="small8", tag="small8")
                nc.vector.tensor_copy(small8[:, 0:NCC], mug)
                nc.vector.tensor_copy(small8[:, NCC:2 * NCC], rstdg)
                # broadcast groups -> 128 partitions
                nc.tensor.matmul(bc_ps, lhsT=hm_sb, rhs=small8, start=True, stop=True)
                bc_sb = gnsb.tile([P, 2 * NCC], F32, name="bc_sb", tag="bc_sb")
                nc.vector.tensor_copy(bc_sb, bc_ps)
                mu_col = bc_sb[:, 0:NCC]
                a_col = bc_sb[:, NCC:2 * NCC]
                if gn_affine:
                    nc.vector.tensor_mul(a_col, a_col, gam_t)
                for cc in range(NCC):
                    nc.vector.tensor_scalar(
                        out=xn_sb[cc], in0=x_sb[cc],
                        scalar1=mu_col[:, cc:cc + 1], scalar2=a_col[:, cc:cc + 1],
                        op0=OP.subtract, op1=OP.mult)
                    if gn_affine:
                        # += beta (beta needs rescale-free add after scale):
                        # xn = xn + beta ; beta broadcast per partition scalar
                        nc.vector.tensor_scalar(
                            out=xn_sb[cc], in0=xn_sb[cc],
                            scalar1=bet_t[:, cc:cc + 1], scalar2=None,
                            op0=OP.add)

            # ================= QKV projections =================
            with tc.tile_pool(name="proj_ps", bufs=2, space="PSUM") as pps, \
                 tc.tile_pool(name="vproj_ps", bufs=2, space="PSUM") as vps:

                def qk_proj(cot, wsb, dst, bname):
                    ps = pps.tile([P, T], F32, name=f"qk_ps{cot}", tag="qk")
                    for half in range(2):
                        o = ps[:, ts(half, 512)]
                        for cc in range(NCC):
                            nc.tensor.matmul(
                                o, lhsT=wsb[cc][:, ts(cot, P)],
                                rhs=xn_sb[cc][:, ts(half, 512)],
                                start=(cc == 0), stop=(cc == NCC - 1 and bname is None))
                        if bname is not None:
                            nc.tensor.matmul(
                                o, lhsT=bias_sb[bname][0:1, ts(cot, P)],
                                rhs=onesrow[0:1, ts(half, 512)],
                                start=False, stop=True)
                    nc.vector.tensor_copy(dst[cot], ps)

                def v_proj(tt):
                    ps = vps.tile([P, C], F32, name=f"v_ps{tt}", tag="v")
                    for cc in range(NCC):
                        nc.tensor.matmul(ps, lhsT=xn_sb[cc][:, ts(tt, P)], rhs=wv_sb[cc],
                                         start=(cc == 0), stop=(cc == NCC - 1 and not vb))
                    if vb:
                        nc.tensor.matmul(ps, lhsT=onesrow[0:1, ts(tt, P)],
                                         rhs=bias_sb["bv"][0:1, :], start=False, stop=True)
                    nc.vector.tensor_copy(v_sb[tt], ps)

                qk_proj(0, wq_sb, qT_sb, "bq" if qb else None)
                qk_proj(0, wk_sb, kT_sb, "bk" if kb else None)
                for tt in range(NTT):
                    v_proj(tt)
                for cot in range(1, NCC):
                    qk_proj(cot, wq_sb, qT_sb, "bq" if qb else None)
                    qk_proj(cot, wk_sb, kT_sb, "bk" if kb else None)

            # ================= attention (4 head-pairs) =================
            with tc.tile_pool(name="sc_ps", bufs=2, space="PSUM") as scps, \
                 tc.tile_pool(name="ctx_ps", bufs=1, space="PSUM") as ctxps, \
                 tc.tile_pool(name="sum_ps", bufs=1, space="PSUM") as sumps, \
                 tc.tile_pool(name="probs", bufs=24) as prpool, \
                 tc.tile_pool(name="rspool", bufs=2) as rspool:
                for p in range(NCC):  # head pair p = heads (2p, 2p+1)
                    # scores + exp: probsT[j, i] per head, 8 j-tiles each
                    prs = [[None, None] for _ in range(NTT)]
                    for jt in range(NTT):
                        for h2 in range(2):
                            rows = slice(64 * h2, 64 * h2 + 64)
                            sc = scps.tile([P, T], F32, name=f"sc{p}_{jt}_{h2}", tag="sc")
                            for half in range(2):
                                nc.tensor.matmul(
                                    sc[:, ts(half, 512)],
                                    lhsT=kT_sb[p][rows, ts(jt, P)],
                                    rhs=qT_sb[p][rows, ts(half, 512)],
                                    start=True, stop=True)
                            pr = prpool.tile([P, T], BF16, name=f"pr{p}_{jt}_{h2}", tag="pr")
                            nc.scalar.activation(pr, sc, AF.Exp, scale=SCALE)
                            prs[jt][h2] = pr
                    # ctx^T and softmax sums via accumulating matmuls.
                    # Two "waves" so each PSUM bank hosts exactly one
                    # accumulation chain at a time, while the two chains of a
                    # wave sit in different banks/col-groups (concurrent).
                    ctx = ctxps.tile([P, T], F32, name=f"ctx{p}", tag="ctx")
                    smm = sumps.tile([P, T], F32, name=f"smm{p}", tag="smm")
                    for wave in range(2):
                        for jt in range(NTT):
                            st, sp = jt == 0, jt == NTT - 1
                            for h2 in range(2):
                                half = (h2 + wave) % 2
                                rows = slice(64 * h2, 64 * h2 + 64)
                                voff = P * p + 64 * h2
                                pr = prs[jt][h2]
                                nc.tensor.matmul(
                                    ctx[rows, ts(half, 512)],
                                    lhsT=v_sb[jt][:, voff:voff + 64],
                                    rhs=pr[:, ts(half, 512)],
                                    start=st, stop=sp)
                                nc.tensor.matmul(
                                    smm[rows, ts(half, 512)],
                                    lhsT=ones64,
                                    rhs=pr[:, ts(half, 512)],
                                    start=st, stop=sp)
                    rs = rspool.tile([P, T], F32, name=f"rs{p}", tag="rs")
                    nc.vector.reciprocal_approx_fast(out=rs, in_=smm)
                    nc.vector.tensor_mul(ctxT_sb[p], ctx, rs)

            # ================= o_proj + LayerNorm + transpose + residual =================
            with tc.tile_pool(name="o_ps", bufs=2, space="PSUM") as ops_, \
                 tc.tile_pool(name="tp_ps", bufs=4, space="PSUM") as tpps, \
                 tc.tile_pool(name="ln_sb", bufs=4) as lnsb, \
                 tc.tile_pool(name="oln_sb", bufs=2) as olnp:
                for it in range(NTT):
                    o_ps = ops_.tile([P, C], F32, name=f"ops{it}", tag="o")
                    for p in range(NCC):
                        nc.tensor.matmul(o_ps, lhsT=ctxT_sb[p][:, ts(it, P)], rhs=wo_sb[p],
                                         start=(p == 0), stop=(p == NCC - 1 and not ob))
                    if ob:
                        nc.tensor.matmul(o_ps, lhsT=onesrow[0:1, ts(it, P)],
                                         rhs=bias_sb["bo"][0:1, :], start=False, stop=True)
                    bnl = lnsb.tile([P, 6], F32, name=f"bnl{it}", tag="bnl")
                    nc.vector.bn_stats(out=bnl, in_=o_ps)
                    mvl = lnsb.tile([P, 2], F32, name=f"mvl{it}", tag="mvl")
                    nc.vector.bn_aggr(out=mvl, in_=bnl)
                    lnv2 = lnsb.tile([P, 1], F32, name=f"lnv2_{it}", tag="lnv2")
                    nc.scalar.activation(lnv2, mvl[:, 1:2], AF.Ln, bias=eps128)
                    rst = lnsb.tile([P, 1], F32, name=f"rst{it}", tag="rst")
                    nc.scalar.activation(rst, lnv2, AF.Exp, scale=-0.5)
                    oln = olnp.tile([P, C], F32, name=f"oln{it}", tag="oln")
                    nc.vector.tensor_scalar(
                        out=oln, in0=o_ps, scalar1=mvl[:, 0:1], scalar2=rst,
                        op0=OP.subtract, op1=OP.mult)
                    if ln_affine:
                        nc.vector.tensor_mul(oln, oln, lng_bc)
                        nc.vector.tensor_add(oln, oln, lnb_bc)
                    for ct in range(NCC):
                        tp = tpps.tile([P, P], F32, name=f"tp{it}_{ct}", tag="tp")
                        nc.tensor.transpose(tp, oln[:, ts(ct, P)], id_sb)
                        nc.vector.tensor_add(out_sb[ct][:, ts(it, P)], tp,
                                             x_sb[ct][:, ts(it, P)])
            for ct in range(NCC):
                nc.sync.dma_start(out=out_d[ts(ct, P), :], in_=out_sb[ct])

    nc.compile()
    return nc


@functools.lru_cache(maxsize=4)
def _build_cached(flags):
    return _build(*flags)


def _prepare(inputs):
    hs = np.asarray(inputs["hidden_states"], dtype=np.float32)
    assert hs.shape == (B, C, Hh, Ww)

    def nontriv(v, ref):
        return not np.all(np.asarray(v) == ref)

    gn_affine = nontriv(inputs["gn_gamma"], 1.0) or nontriv(inputs["gn_beta"], 0.0)
    ln_affine = nontriv(inputs["ln_gamma"], 1.0) or nontriv(inputs["ln_beta"], 0.0)
    qb = nontriv(inputs["bq"], 0.0)
    kb = nontriv(inputs["bk"], 0.0)
    vb = nontriv(inputs["bv"], 0.0)
    ob = nontriv(inputs["bo"], 0.0)
    flags = (gn_affine, qb, kb, vb, ob, ln_affine)

    bf = ml_dtypes.bfloat16
    shared = {
        "wqT": np.ascontiguousarray(np.asarray(inputs["wq"], np.float32).T).astype(bf),
        "wkT": np.ascontiguousarray(np.asarray(inputs["wk"], np.float32).T).astype(bf),
        "wvT": np.ascontiguousarray(np.asarray(inputs["wv"], np.float32).T).astype(bf),
        "woT": np.ascontiguousarray(np.asarray(inputs["wo"], np.float32).T).astype(bf),
        "g8": (np.arange(P)[:, None] // GSZ == np.arange(8)[None, :]).astype(np.float32) / GSZ,
        "hm": (np.arange(P)[None, :] // GSZ == np.arange(8)[:, None]).astype(np.float32),
        "ident": np.eye(P, dtype=np.float32),
    }
    if gn_affine:
        shared["gn_gamma"] = np.asarray(inputs["gn_gamma"], np.float32)
        shared["gn_beta"] = np.asarray(inputs["gn_beta"], np.float32)
    if ln_affine:
        shared["ln_gamma"] = np.asarray(inputs["ln_gamma"], np.float32)
        shared["ln_beta"] = np.asarray(inputs["ln_beta"], np.float32)
    for name, flag in (("bq", qb), ("bk", kb), ("bv", vb), ("bo", ob)):
        if flag:
            shared[name] = np.asarray(inputs[name], np.float32).reshape(1, C).astype(bf)

    in_maps = [dict(shared, x=np.ascontiguousarray(hs[i].reshape(C, T)))
               for i in range(B)]
    return flags, in_maps


def kernel(**inputs) -> np.ndarray:
    global _last_result
    flags, in_maps = _prepare(inputs)
    nc = _build_cached(flags)
    res = bass_utils.run_bass_kernel_spmd(nc, in_maps, core_ids=list(range(B)))
    _last_result = res
    out = np.stack([res.results[i]["out"].reshape(C, Hh, Ww) for i in range(B)])
    return out
